# revision 2
# baseline (speedup 1.0000x reference)
"""Single-directional Chamfer distance (pytorch3d semantics) on 8 trn2 cores.

loss = mean_b mean_i min_j ||x_bi - y_bj||^2   with x = v_pred, y = v.

Sharding: batch B=8 across the 8 cores, one point-cloud pair per core.

Per-core algorithm (all-pairs, bf16 split-precision matmul):
  d2[i,j] = xsq_i + ysq_j - 2 x_i.y_j computed DIRECTLY in PSUM by a K=16
  bf16 matmul over augmented coordinates, with every fp32 operand split
  into (hi, lo) bf16 parts so each partial product is exact in the PE's
  fp32 accumulator:
     xsq = xsqh + xsql   paired against (1, 1)
     ysq = ysqh + ysql   paired against (1, 1)
     -2x = mh + ml, y = yh + yl -> 4 cross terms mh.yh mh.yl ml.yh ml.yl
  Residual error ~2^-18 relative per operand -- ~1e-5 absolute in d2,
  vs bf16's 2^-9 which would be ~6e-3.  bf16 matmuls stream 1 col/cycle
  (vs 4 for fp32): 4x faster PE.

  min_j d2 via DVE tensor_tensor_reduce over the two halves of each
  [128, 2048] PSUM tile: one instruction streams 2048 elems/half through
  both read ports (1024 cycles), i.e. 2x the rate of tensor_reduce.
  Running per-query-block min chains through the accum scalar.

Raw bass implementation (explicit semaphores): PE/DVE ping-pong over two
4-bank PSUM buffers, one semaphore each way.
"""

import os

import numpy as np
from ml_dtypes import bfloat16

import concourse.bass as bass
import concourse.mybir as mybir
from concourse.bass_utils import run_bass_kernel_spmd

F32 = mybir.dt.float32
BF16 = mybir.dt.bfloat16
N = 16384
NCORES = 8
KAUG = 16

TBS = 2048           # targets per PSUM tile (4 banks)
HTBS = TBS // 2
NTB = N // TBS       # 8
NQB = N // 128       # 128 query blocks
NT = NQB * NTB       # 1024 psum tiles

_BUILD_CACHE = {}


def _build():
    nc = bass.Bass()
    xa = nc.dram_tensor("xa", [KAUG, N], BF16, kind="ExternalInput")
    ya = nc.dram_tensor("ya", [KAUG, N], BF16, kind="ExternalInput")
    out = nc.dram_tensor("out", [1, 128], F32, kind="ExternalOutput")

    AX = mybir.AxisListType
    OP = mybir.AluOpType
    BIG = 3.0e38

    with (
        nc.sbuf_tensor([KAUG, N], BF16) as lhsT,
        nc.sbuf_tensor([KAUG, N], BF16) as rhs,
        nc.sbuf_tensor([128, HTBS], F32) as scratch,
        nc.sbuf_tensor([128, NQB], F32) as m_grid,
        nc.sbuf_tensor([128, 1], F32) as s_m,
        nc.psum_tensor([128, TBS], F32) as psA,
        nc.psum_tensor([128, TBS], F32) as psB,
        nc.semaphore() as dma_sem,
        nc.semaphore() as mm_sem,
        nc.semaphore() as red_sem,
        nc.semaphore() as dve_done,
        nc.Block() as block,
    ):

        @block.sync
        def _(sync):
            sync.dma_start(lhsT[:, :], xa[:, :]).then_inc(dma_sem, 16)
            sync.dma_start(rhs[:, :], ya[:, :]).then_inc(dma_sem, 16)
            sync.wait_ge(dve_done, 1)
            sync.dma_start(out[0:1, :].rearrange("a b -> b a"), s_m[:, :]).then_inc(dma_sem, 16)

        @block.tensor
        def _(tensor):
            tensor.wait_ge(dma_sem, 32)
            for tidx in range(NT):
                qb, tb = divmod(tidx, NTB)
                ps = psA if tidx % 2 == 0 else psB
                if tidx >= 2:
                    # wait until the reduce of tile tidx-2 released this
                    # psum buffer (red_sem counts finished reduces)
                    tensor.wait_ge(red_sem, tidx - 1)
                lw = lhsT[:, qb * 128 : (qb + 1) * 128]
                for k in range(TBS // 512):
                    c0 = tb * TBS + k * 512
                    mm = nc.tensor.matmul(
                        ps[:, k * 512 : (k + 1) * 512],
                        lw,
                        rhs[:, c0 : c0 + 512],
                        start=True,
                        stop=True,
                    )
                mm.then_inc(mm_sem, 1)

        @block.vector
        def _(vector):
            for tidx in range(NT):
                qb, tb = divmod(tidx, NTB)
                ps = psA if tidx % 2 == 0 else psB
                vector.wait_ge(mm_sem, tidx + 1)
                # one DVE instruction min-reduces the whole 2048-wide tile:
                # elementwise min of the two 1024 halves (2 read ports),
                # reduce-min of that, chained through m_grid[:, qb]
                nc.vector.tensor_tensor_reduce(
                    out=scratch[:, :],
                    in0=ps[:, 0:HTBS],
                    in1=ps[:, HTBS:TBS],
                    scale=1.0,
                    scalar=(BIG if tb == 0 else m_grid[:, qb : qb + 1]),
                    op0=OP.min,
                    op1=OP.min,
                    accum_out=m_grid[:, qb : qb + 1],
                ).then_inc(red_sem, 1)
            nc.vector.tensor_reduce(
                s_m[:, :], m_grid[:, :], axis=AX.X, op=OP.add
            ).then_inc(dve_done, 1)

    return nc


def _split(a):
    hi = a.astype(bfloat16)
    lo = (a - hi.astype(np.float64)).astype(bfloat16)
    return hi, lo


def _marshal(v: np.ndarray, v_pred: np.ndarray):
    """Host-side operand marshalling: transposes, norms, and (hi, lo) bf16
    splits of the augmented coordinates for the split-precision matmul."""
    in_maps = []
    for b in range(NCORES):
        x64 = v_pred[b].astype(np.float64)
        y64 = v[b].astype(np.float64)
        xsqh, xsql = _split((x64 * x64).sum(axis=1))
        ysqh, ysql = _split((y64 * y64).sum(axis=1))
        mh, ml = _split(-2.0 * x64)
        yh, yl = _split(y64)

        xa = np.empty((KAUG, N), bfloat16)
        xa[0] = xsqh
        xa[1] = xsql
        xa[2] = 1.0
        xa[3] = 1.0
        xa[4:7] = mh.T
        xa[7:10] = mh.T
        xa[10:13] = ml.T
        xa[13:16] = ml.T

        ya = np.empty((KAUG, N), bfloat16)
        ya[0] = 1.0
        ya[1] = 1.0
        ya[2] = ysqh
        ya[3] = ysql
        ya[4:7] = yh.T
        ya[7:10] = yl.T
        ya[10:13] = yh.T
        ya[13:16] = yl.T

        in_maps.append({"xa": xa, "ya": ya})
    return in_maps


def kernel(v: np.ndarray, v_pred: np.ndarray) -> np.ndarray:
    v = np.ascontiguousarray(np.asarray(v, dtype=np.float32))
    v_pred = np.ascontiguousarray(np.asarray(v_pred, dtype=np.float32))
    assert v.shape == (NCORES, N, 3) and v_pred.shape == (NCORES, N, 3)

    if "k" not in _BUILD_CACHE:
        _BUILD_CACHE["k"] = _build()
    nc = _BUILD_CACHE["k"]

    in_maps = _marshal(v, v_pred)
    res = run_bass_kernel_spmd(
        nc,
        in_maps,
        core_ids=list(range(NCORES)),
        trace=bool(int(os.environ.get("BASS_TRACE_KERNEL", "0"))),
    )
    if res.exec_time_ns is not None:
        print(f"HW exec time: {res.exec_time_ns} ns")

    per_core = []
    for r in res.results:
        o = np.asarray(r["out"], dtype=np.float64)
        per_core.append(o.sum() / N)
    loss = np.float32(np.mean(per_core))
    return np.array(loss, dtype=np.float32)
